# revision 4
# baseline (speedup 1.0000x reference)
"""LSH-masked linear layer (LSHLinearStrided) on 8 trn2 NeuronCores.

Computation (see problem reference):
    code_x = simhash(x, proj)   [B,S,T]    code_w = simhash(W, proj)  [O,T]
    mask[b,s,o] = any_t(code_x[...,t] == code_w[o,t])
    out = where(mask, x @ W.T + b, 0)

Strategy:
  - Hash codes are sign decisions on dot products; recomputing them with a
    different accumulation order flips borderline bits and each flip costs
    ~5e-4 global rel-err. So the codes are computed with the exact same jnp
    ops as the reference (same XLA program on the same default device ->
    bit-identical), then turned into one-hot matrices.
  - Everything heavy runs on the NeuronCores, data-parallel over the 8192
    tokens (1024 tokens/core):
      * main GEMM x @ W.T as a 3-term bf16 split (hi/lo), fp32-grade
        (~4e-6 rel err), at full bf16 PE throughput
      * mask via a small one-hot GEMM: cnt = U_x @ U_w.T (exact integer
        counts), K = T*64 = 512, in fp8e4m3 with DoubleRow packing
        (0/1 and counts <= 8 are exact)
      * fused epilogue on DVE: out = (cnt > 0.5) * (xW + b)
  - Loop: n-outer (8 slices of 512 neurons), m-inner (8 tiles of 128
    tokens). x stays SBUF-resident (loaded per-m so compute starts early);
    W slices stream (read exactly once).
"""

import os
import sys
import types
from contextlib import ExitStack

import numpy as np
import ml_dtypes

import concourse.bass as bass
import concourse.tile as tile
from concourse import bacc, mybir
from concourse.bass_utils import run_bass_kernel_spmd

BF16 = ml_dtypes.bfloat16
FP8 = ml_dtypes.float8_e4m3

B, S, D, O, T, HB = 4, 2048, 1024, 4096, 8, 6
N_CORES = 8
BS = B * S                 # 8192 tokens
TOK = BS // N_CORES        # 1024 tokens per core
C = T * (2 ** HB)          # 512 one-hot hash dim
M_TILES = TOK // 128       # 8
N_TILES = O // 512         # 8
K_TILES = D // 128         # 8
C_TILES = C // 128         # 4

LAST_EXEC_NS = None
_PROG = None


def _install_ntff_hook():
    """Restore the NTFF profile hook that trn_boot skips when
    antenv.axon_hooks is absent. Only needed when tracing (BASS_TRACE=1)."""
    if "antenv.axon_hooks" in sys.modules:
        return
    try:
        import antenv

        hooks = types.ModuleType("antenv.axon_hooks")
        _h = [None]
        hooks.set_axon_ntff_profile_hook = lambda h: _h.__setitem__(0, h)
        hooks.get_axon_ntff_profile_hook = lambda: _h[0]
        sys.modules["antenv.axon_hooks"] = hooks
        antenv.axon_hooks = hooks
        from trn_agent_boot.trn_boot import _ntff_profile_via_ctypes

        hooks.set_axon_ntff_profile_hook(
            _ntff_profile_via_ctypes("/opt/axon/libaxon_pjrt.so")
        )
    except Exception:
        pass


def _hash_codes_like_reference(v, proj):
    """Bit-identical replica of the reference's _hash_codes."""
    import jax.numpy as jnp

    bits = jnp.einsum('...d,thd->...th', v, proj) > 0
    H = proj.shape[1]
    weights = (2 ** jnp.arange(H)).astype(jnp.int32)
    return np.asarray(jnp.sum(bits.astype(jnp.int32) * weights, axis=-1))


def _one_hot_T(codes, n_items):
    """codes [n_items, T] int -> transposed one-hot [C, n_items] fp8."""
    u = np.zeros((n_items, C), dtype=FP8)
    cols = codes + (np.arange(T, dtype=np.int32) * 64)[None, :]
    u[np.arange(n_items)[:, None], cols] = FP8(1.0)
    return np.ascontiguousarray(u.T)


def _build_program():
    nc = bacc.Bacc("TRN2", target_bir_lowering=False, debug=False,
                   num_devices=N_CORES)
    dt = mybir.dt

    # Per-core inputs: x.T hi/lo splits [D, TOK], one-hot codes [C, TOK].
    xhiT = nc.dram_tensor("xhiT", [D, TOK], dt.bfloat16, kind="ExternalInput").ap()
    xloT = nc.dram_tensor("xloT", [D, TOK], dt.bfloat16, kind="ExternalInput").ap()
    uxT = nc.dram_tensor("uxT", [C, TOK], dt.float8e4, kind="ExternalInput").ap()
    # Shared inputs: W.T hi/lo [D, O], one-hot W codes [C, O], bias bcast.
    whiT = nc.dram_tensor("whiT", [D, O], dt.bfloat16, kind="ExternalInput").ap()
    wloT = nc.dram_tensor("wloT", [D, O], dt.bfloat16, kind="ExternalInput").ap()
    uwT = nc.dram_tensor("uwT", [C, O], dt.float8e4, kind="ExternalInput").ap()
    biasb = nc.dram_tensor("biasb", [128, O], dt.float32, kind="ExternalInput").ap()
    out = nc.dram_tensor("out", [TOK, O], dt.float32, kind="ExternalOutput").ap()

    xhiT_k = xhiT.rearrange("(k p) t -> p k t", p=128)   # [128, 8, 1024]
    xloT_k = xloT.rearrange("(k p) t -> p k t", p=128)
    uxT_k = uxT.rearrange("(c p) t -> p c t", p=128)     # [128, 4, 1024]

    with tile.TileContext(nc) as tc, ExitStack() as ctx:
        resident = ctx.enter_context(tc.tile_pool(name="resident", bufs=1))
        wpool = ctx.enter_context(tc.tile_pool(name="wpool", bufs=2))
        temps = ctx.enter_context(tc.tile_pool(name="temps", bufs=10))
        outp = ctx.enter_context(tc.tile_pool(name="outp", bufs=10))
        psum_main = ctx.enter_context(
            tc.tile_pool(name="psum_main", bufs=4, space="PSUM"))
        psum_cnt = ctx.enter_context(
            tc.tile_pool(name="psum_cnt", bufs=3, space="PSUM"))

        def load_w(n):
            ns = bass.ts(n, 512)
            wn = {}
            for name, src, kt, dtt in (
                    ("whi", whiT, K_TILES, dt.bfloat16),
                    ("wlo", wloT, K_TILES, dt.bfloat16),
                    ("uw", uwT, C_TILES, dt.float8e4)):
                t = wpool.tile([128, kt, 512], dtt, tag=name)
                src_k = src.rearrange("(k p) o -> p k o", p=128)
                nc.sync.dma_start(t[:], src_k[:, :, ns])
                wn[name] = t
            return wn

        # First n-slice of W goes first so main matmuls unblock early.
        wn0 = load_w(0)

        # Resident x, per-m tiles so the first matmul group unblocks early.
        xs = {"xhi": [], "xlo": []}
        ux_sb = []

        def load_x(m):
            ms = bass.ts(m, 128)
            for name, src in (("xhi", xhiT_k), ("xlo", xloT_k)):
                t = resident.tile([128, K_TILES, 128], dt.bfloat16,
                                  tag=f"{name}_{m}")
                nc.sync.dma_start(t[:], src[:, :, ms])
                xs[name].append(t)
            t = resident.tile([128, C_TILES, 128], dt.float8e4, tag=f"ux_{m}")
            nc.sync.dma_start(t[:], uxT_k[:, :, ms])
            ux_sb.append(t)

        load_x(0)
        load_x(1)
        wn1 = load_w(1)  # prefetch before the bulk of x lands
        for m in range(2, M_TILES):
            load_x(m)
        bias_sb = resident.tile([128, O], dt.float32, tag="bias")
        nc.sync.dma_start(bias_sb[:], biasb[:])

        for n in range(N_TILES):
            ns = bass.ts(n, 512)
            # Stream this n-iteration's weight slices (read exactly once).
            wn = wn0 if n == 0 else (wn1 if n == 1 else load_w(n))

            for m in range(M_TILES):
                ms = bass.ts(m, 128)
                # Count GEMM: fp8 DoubleRow packs two 128-K slabs per pass.
                pc = psum_cnt.tile([128, 512], dt.float32, tag="pc")
                for c2 in range(C_TILES // 2):
                    nc.tensor.matmul(
                        pc[:], ux_sb[m][:, 2 * c2:2 * c2 + 2, :],
                        wn["uw"][:, 2 * c2:2 * c2 + 2, :],
                        start=(c2 == 0), stop=(c2 == C_TILES // 2 - 1),
                        perf_mode=mybir.MatmulPerfMode.DoubleRow)
                pm = psum_main.tile([128, 512], dt.float32, tag="pm")
                terms = (("xhi", "whi"), ("xhi", "wlo"), ("xlo", "whi"))
                n_mm = len(terms) * K_TILES
                i = 0
                for xn, wn_name in terms:
                    for k in range(K_TILES):
                        nc.tensor.matmul(pm[:], xs[xn][m][:, k, :],
                                         wn[wn_name][:, k, :],
                                         start=(i == 0), stop=(i == n_mm - 1))
                        i += 1
                # Epilogue: out = (cnt > 0.5) * (xW + b)
                tmp = temps.tile([128, 512], dt.float32, tag="tmp")
                nc.vector.tensor_tensor(tmp[:], pm[:], bias_sb[:, ns],
                                        mybir.AluOpType.add)
                ot = outp.tile([128, 512], dt.float32, tag="ot")
                nc.vector.scalar_tensor_tensor(
                    ot[:], pc[:], 0.5, tmp[:],
                    mybir.AluOpType.is_gt, mybir.AluOpType.mult)
                nc.sync.dma_start(out[ms, ns], ot[:])

    nc.compile()
    return nc


def _split_hi_lo(a32):
    hi = a32.astype(BF16)
    lo = (a32 - hi.astype(np.float32)).astype(BF16)
    return hi, lo


def kernel(x, W, b, proj):
    global LAST_EXEC_NS, _PROG

    x = np.asarray(x, dtype=np.float32)
    W = np.asarray(W, dtype=np.float32)
    b = np.asarray(b, dtype=np.float32)
    proj = np.asarray(proj, dtype=np.float32)

    # Hash codes, bit-identical to the reference.
    code_x = _hash_codes_like_reference(x, proj).reshape(BS, T)
    code_w = _hash_codes_like_reference(W, proj)

    uxT_full = _one_hot_T(code_x, BS)        # [C, BS] fp8
    uwT = _one_hot_T(code_w, O)              # [C, O] fp8

    WT = np.ascontiguousarray(W.T)           # [D, O]
    whiT, wloT = _split_hi_lo(WT)
    biasb = np.ascontiguousarray(np.broadcast_to(b, (128, O)))

    xT_full = np.ascontiguousarray(x.reshape(BS, D).T)  # [D, BS]
    xhiT_full, xloT_full = _split_hi_lo(xT_full)

    if _PROG is None:
        _PROG = _build_program()

    in_maps = []
    for c in range(N_CORES):
        ts = slice(c * TOK, (c + 1) * TOK)
        in_maps.append({
            "xhiT": np.ascontiguousarray(xhiT_full[:, ts]),
            "xloT": np.ascontiguousarray(xloT_full[:, ts]),
            "uxT": np.ascontiguousarray(uxT_full[:, ts]),
            "whiT": whiT, "wloT": wloT, "uwT": uwT, "biasb": biasb,
        })

    trace = bool(os.environ.get("BASS_TRACE"))
    if trace:
        _install_ntff_hook()
    res = run_bass_kernel_spmd(_PROG, in_maps, list(range(N_CORES)),
                               trace=trace)
    LAST_EXEC_NS = res.exec_time_ns

    out = np.concatenate([res.results[c]["out"] for c in range(N_CORES)],
                         axis=0)
    return out.reshape(B, S, O)


# revision 5
# speedup vs baseline: 1.0159x; 1.0159x over previous
"""LSH-masked linear layer (LSHLinearStrided) on 8 trn2 NeuronCores.

Computation (see problem reference):
    code_x = simhash(x, proj)   [B,S,T]    code_w = simhash(W, proj)  [O,T]
    mask[b,s,o] = any_t(code_x[...,t] == code_w[o,t])
    out = where(mask, x @ W.T + b, 0)

Strategy:
  - Hash codes are sign decisions on dot products; recomputing them with a
    different accumulation order flips borderline bits and each flip costs
    ~5e-4 global rel-err. So the codes are computed with the exact same jnp
    ops as the reference (same XLA program on the same default device ->
    bit-identical), then turned into one-hot matrices.
  - Everything heavy runs on the NeuronCores, data-parallel over the 8192
    tokens (1024 tokens/core):
      * main GEMM x @ W.T as a 3-term bf16 split (hi/lo), fp32-grade
        (~4e-6 rel err), at full bf16 PE throughput
      * mask via a small one-hot GEMM: cnt = U_x @ U_w.T (exact integer
        counts), K = T*64 = 512, in fp8e4m3 with DoubleRow packing
        (0/1 and counts <= 8 are exact)
      * fused epilogue on DVE: out = (cnt > 0.5) * (xW + b)
  - Loop: n-outer (8 slices of 512 neurons), m-inner (8 tiles of 128
    tokens). x stays SBUF-resident (loaded per-m so compute starts early);
    W slices stream (read exactly once).
"""

import os
import sys
import types
from contextlib import ExitStack

import numpy as np
import ml_dtypes

import concourse.bass as bass
import concourse.tile as tile
from concourse import bacc, mybir
from concourse.bass_utils import run_bass_kernel_spmd

BF16 = ml_dtypes.bfloat16
FP8 = ml_dtypes.float8_e4m3

B, S, D, O, T, HB = 4, 2048, 1024, 4096, 8, 6
N_CORES = 8
BS = B * S                 # 8192 tokens
TOK = BS // N_CORES        # 1024 tokens per core
C = T * (2 ** HB)          # 512 one-hot hash dim
M_TILES = TOK // 128       # 8
N_TILES = O // 512         # 8
K_TILES = D // 128         # 8
C_TILES = C // 128         # 4

LAST_EXEC_NS = None
_PROG = None


def _install_ntff_hook():
    """Restore the NTFF profile hook that trn_boot skips when
    antenv.axon_hooks is absent. Only needed when tracing (BASS_TRACE=1)."""
    if "antenv.axon_hooks" in sys.modules:
        return
    try:
        import antenv

        hooks = types.ModuleType("antenv.axon_hooks")
        _h = [None]
        hooks.set_axon_ntff_profile_hook = lambda h: _h.__setitem__(0, h)
        hooks.get_axon_ntff_profile_hook = lambda: _h[0]
        sys.modules["antenv.axon_hooks"] = hooks
        antenv.axon_hooks = hooks
        from trn_agent_boot.trn_boot import _ntff_profile_via_ctypes

        hooks.set_axon_ntff_profile_hook(
            _ntff_profile_via_ctypes("/opt/axon/libaxon_pjrt.so")
        )
    except Exception:
        pass


def _hash_codes_like_reference(v, proj):
    """Bit-identical replica of the reference's _hash_codes."""
    import jax.numpy as jnp

    bits = jnp.einsum('...d,thd->...th', v, proj) > 0
    H = proj.shape[1]
    weights = (2 ** jnp.arange(H)).astype(jnp.int32)
    return np.asarray(jnp.sum(bits.astype(jnp.int32) * weights, axis=-1))


def _one_hot_T(codes, n_items):
    """codes [n_items, T] int -> transposed one-hot [C, n_items] fp8."""
    u = np.zeros((n_items, C), dtype=FP8)
    cols = codes + (np.arange(T, dtype=np.int32) * 64)[None, :]
    u[np.arange(n_items)[:, None], cols] = FP8(1.0)
    return np.ascontiguousarray(u.T)


def _build_program():
    nc = bacc.Bacc("TRN2", target_bir_lowering=False, debug=False,
                   num_devices=N_CORES)
    dt = mybir.dt

    # Per-core inputs: x.T hi/lo splits [D, TOK], one-hot codes [C, TOK].
    xhiT = nc.dram_tensor("xhiT", [D, TOK], dt.bfloat16, kind="ExternalInput").ap()
    xloT = nc.dram_tensor("xloT", [D, TOK], dt.bfloat16, kind="ExternalInput").ap()
    uxT = nc.dram_tensor("uxT", [C, TOK], dt.float8e4, kind="ExternalInput").ap()
    # Shared inputs: W.T hi/lo [D, O], one-hot W codes [C, O], bias bcast.
    whiT = nc.dram_tensor("whiT", [D, O], dt.bfloat16, kind="ExternalInput").ap()
    wloT = nc.dram_tensor("wloT", [D, O], dt.bfloat16, kind="ExternalInput").ap()
    uwT = nc.dram_tensor("uwT", [C, O], dt.float8e4, kind="ExternalInput").ap()
    biasb = nc.dram_tensor("biasb", [128, O], dt.float32, kind="ExternalInput").ap()
    out = nc.dram_tensor("out", [TOK, O], dt.float32, kind="ExternalOutput").ap()

    xhiT_k = xhiT.rearrange("(k p) t -> p k t", p=128)   # [128, 8, 1024]
    xloT_k = xloT.rearrange("(k p) t -> p k t", p=128)
    uxT_k = uxT.rearrange("(c p) t -> p c t", p=128)     # [128, 4, 1024]

    with tile.TileContext(nc) as tc, ExitStack() as ctx:
        resident = ctx.enter_context(tc.tile_pool(name="resident", bufs=1))
        wpool = ctx.enter_context(tc.tile_pool(name="wpool", bufs=2))
        temps = ctx.enter_context(tc.tile_pool(name="temps", bufs=10))
        outp = ctx.enter_context(tc.tile_pool(name="outp", bufs=10))
        psum_main = ctx.enter_context(
            tc.tile_pool(name="psum_main", bufs=4, space="PSUM"))
        psum_cnt = ctx.enter_context(
            tc.tile_pool(name="psum_cnt", bufs=3, space="PSUM"))

        def load_w(n):
            ns = bass.ts(n, 512)
            wn = {}
            for name, src, kt, dtt in (
                    ("whi", whiT, K_TILES, dt.bfloat16),
                    ("wlo", wloT, K_TILES, dt.bfloat16),
                    ("uw", uwT, C_TILES, dt.float8e4)):
                t = wpool.tile([128, kt, 512], dtt, tag=name)
                src_k = src.rearrange("(k p) o -> p k o", p=128)
                nc.sync.dma_start(t[:], src_k[:, :, ns])
                wn[name] = t
            return wn

        # First n-slice of W goes first so main matmuls unblock early.
        wn0 = load_w(0)

        # Resident x, per-m tiles so the first matmul group unblocks early.
        xs = {"xhi": [], "xlo": []}
        ux_sb = []

        def load_x(m):
            ms = bass.ts(m, 128)
            for name, src in (("xhi", xhiT_k), ("xlo", xloT_k)):
                t = resident.tile([128, K_TILES, 128], dt.bfloat16,
                                  tag=f"{name}_{m}")
                nc.sync.dma_start(t[:], src[:, :, ms])
                xs[name].append(t)
            t = resident.tile([128, C_TILES, 128], dt.float8e4, tag=f"ux_{m}")
            nc.sync.dma_start(t[:], uxT_k[:, :, ms])
            ux_sb.append(t)

        for m in range(M_TILES):
            load_x(m)
        bias_sb = resident.tile([128, O], dt.float32, tag="bias")
        nc.sync.dma_start(bias_sb[:], biasb[:])

        for n in range(N_TILES):
            ns = bass.ts(n, 512)
            # Stream this n-iteration's weight slices (read exactly once).
            wn = wn0 if n == 0 else load_w(n)

            for m in range(M_TILES):
                ms = bass.ts(m, 128)
                # Count GEMM: fp8 DoubleRow packs two 128-K slabs per pass.
                pc = psum_cnt.tile([128, 512], dt.float32, tag="pc")
                for c2 in range(C_TILES // 2):
                    nc.tensor.matmul(
                        pc[:], ux_sb[m][:, 2 * c2:2 * c2 + 2, :],
                        wn["uw"][:, 2 * c2:2 * c2 + 2, :],
                        start=(c2 == 0), stop=(c2 == C_TILES // 2 - 1),
                        perf_mode=mybir.MatmulPerfMode.DoubleRow)
                pm = psum_main.tile([128, 512], dt.float32, tag="pm")
                terms = (("xhi", "whi"), ("xhi", "wlo"), ("xlo", "whi"))
                n_mm = len(terms) * K_TILES
                i = 0
                for xn, wn_name in terms:
                    for k in range(K_TILES):
                        nc.tensor.matmul(pm[:], xs[xn][m][:, k, :],
                                         wn[wn_name][:, k, :],
                                         start=(i == 0), stop=(i == n_mm - 1))
                        i += 1
                # Epilogue: out = (cnt > 0.5) * (xW + b)
                tmp = temps.tile([128, 512], dt.float32, tag="tmp")
                nc.vector.tensor_tensor(tmp[:], pm[:], bias_sb[:, ns],
                                        mybir.AluOpType.add)
                ot = outp.tile([128, 512], dt.float32, tag="ot")
                nc.vector.scalar_tensor_tensor(
                    ot[:], pc[:], 0.5, tmp[:],
                    mybir.AluOpType.is_gt, mybir.AluOpType.mult)
                nc.sync.dma_start(out[ms, ns], ot[:])

    nc.compile()
    return nc


def _split_hi_lo(a32):
    hi = a32.astype(BF16)
    lo = (a32 - hi.astype(np.float32)).astype(BF16)
    return hi, lo


def kernel(x, W, b, proj):
    global LAST_EXEC_NS, _PROG

    x = np.asarray(x, dtype=np.float32)
    W = np.asarray(W, dtype=np.float32)
    b = np.asarray(b, dtype=np.float32)
    proj = np.asarray(proj, dtype=np.float32)

    # Hash codes, bit-identical to the reference.
    code_x = _hash_codes_like_reference(x, proj).reshape(BS, T)
    code_w = _hash_codes_like_reference(W, proj)

    uxT_full = _one_hot_T(code_x, BS)        # [C, BS] fp8
    uwT = _one_hot_T(code_w, O)              # [C, O] fp8

    WT = np.ascontiguousarray(W.T)           # [D, O]
    whiT, wloT = _split_hi_lo(WT)
    biasb = np.ascontiguousarray(np.broadcast_to(b, (128, O)))

    xT_full = np.ascontiguousarray(x.reshape(BS, D).T)  # [D, BS]
    xhiT_full, xloT_full = _split_hi_lo(xT_full)

    if _PROG is None:
        _PROG = _build_program()

    in_maps = []
    for c in range(N_CORES):
        ts = slice(c * TOK, (c + 1) * TOK)
        in_maps.append({
            "xhiT": np.ascontiguousarray(xhiT_full[:, ts]),
            "xloT": np.ascontiguousarray(xloT_full[:, ts]),
            "uxT": np.ascontiguousarray(uxT_full[:, ts]),
            "whiT": whiT, "wloT": wloT, "uwT": uwT, "biasb": biasb,
        })

    trace = bool(os.environ.get("BASS_TRACE"))
    if trace:
        _install_ntff_hook()
    res = run_bass_kernel_spmd(_PROG, in_maps, list(range(N_CORES)),
                               trace=trace)
    LAST_EXEC_NS = res.exec_time_ns

    out = np.concatenate([res.results[c]["out"] for c in range(N_CORES)],
                         axis=0)
    return out.reshape(B, S, O)
